# revision 4
# baseline (speedup 1.0000x reference)
"""Trainium2 Bass kernel for GalaxyNetNormalizeOutput.

Math: x is [N, 37]. The reference applies 11 sequential segment
normalizations; the 11 segments exactly partition the 37 columns and each
segment is only written once, so every segment sum is over ORIGINAL values
and the whole op collapses to out[:, j] = x[:, j] * m_{seg(j)} with 11
per-row multipliers that are monomials in the original segment sums
S_k and a few original columns:

  g_k = 1/S_k   (S_k = sum of segment k, step order)
  mA=g0; mB=(g0*x0)*g1; mC=(g0*x1)*g2; mD=(mC*x3)*g3; mE=(mC*x4)*g4
  mF=(mC*x4)*g5; mG=(mF*x7)*g6; mH=(mF*x7)*g7; mI=(mC*x4)*g8
  mJ=g9; mK=(g9*x13)*g10

(the reference's where(s>0) guard never fires for this workload's
uniform-[0,1) data; reciprocals are computed as exp(-ln(s + 1e-30)) on the
scalar engine, optionally refined by one Newton step on DVE.)

Sharding: pure data parallel over rows across 8 NeuronCores; each core runs
an identical SPMD program over its row shard (padded so every core gets
128*B rows).
"""
import sys

for _p in ("/opt/trn_rl_repo", "/root/.axon_site/_ro/trn_rl_repo"):
    if _p not in sys.path:
        sys.path.append(_p)

import numpy as np
import concourse.bass as bass
import concourse.bacc as bacc
import concourse.mybir as mybir
from concourse.tile import TileContext
from concourse.bass_utils import run_bass_kernel_spmd

F32 = mybir.dt.float32
P = 128
NCORES = 8
NCOLS = 37
# segments in STEP order: seg k normalized at step k
SEGS = [(0, 3), (15, 18), (3, 5), (25, 28), (5, 7), (7, 9), (28, 31),
        (31, 37), (9, 13), (13, 15), (18, 25)]
NSEG = 11
MAX_R = 256          # rows per partition per tile
NEWTON = True        # one Newton-Raphson refinement of the ACT reciprocal


def _tile_splits(nblocks):
    """Split nblocks (rows/128 per core) into per-tile rows-per-partition."""
    out = []
    left = nblocks
    while left > 0:
        r = min(MAX_R, left)
        out.append(r)
        left -= r
    return out


def _emit_tile(nc, tc, xpool, spool, x, y, row0, R, bias_tiny):
    mul = mybir.AluOpType.mult
    xt = xpool.tile([P, R * NCOLS], F32, tag="x")
    src = x[row0:row0 + P * R, :].rearrange("(p r) c -> p (r c)", p=P)
    nc.sync.dma_start(xt[:, :], src)

    x3 = xt[:, :].rearrange("p (r c) -> p r c", c=NCOLS)   # [P, R, 37]
    xT = xt[:, :].rearrange("p (r c) -> p c r", c=NCOLS)   # [P, 37, R]

    sums = spool.tile([P, NSEG * R], F32, tag="sums")
    g = spool.tile([P, NSEG * R], F32, tag="g")
    mm = spool.tile([P, 10 * R], F32, tag="mm")

    # 11 per-row segment sums (strided free-dim reduce)
    for k, (s, e) in enumerate(SEGS):
        nc.vector.reduce_sum(sums[:, k * R:(k + 1) * R], x3[:, :, s:e],
                             axis=mybir.AxisListType.X)

    # g = 1/sums via exp(-ln(sums + tiny)) on the scalar engine
    nc.scalar.activation(g[:, :], sums[:, :],
                         mybir.ActivationFunctionType.Ln, bias=bias_tiny[:, :])
    nc.scalar.activation(g[:, :], g[:, :],
                         mybir.ActivationFunctionType.Exp, scale=-1.0)
    if NEWTON:
        # g <- g*(2 - d*g): two fused DVE ops
        u = spool.tile([P, NSEG * R], F32, tag="u")
        nc.vector.scalar_tensor_tensor(u[:, :], sums[:, :], -1.0, g[:, :],
                                       op0=mul, op1=mul)        # u = -d*g
        nc.vector.scalar_tensor_tensor(g[:, :], u[:, :], 2.0, g[:, :],
                                       op0=mybir.AluOpType.add, op1=mul)

    gv = g[:, :].rearrange("p (k r) -> p k r", r=R)      # [P, 11, R]
    mv = mm[:, :].rearrange("p (k r) -> p k r", r=R)     # [P, 10, R]

    def bc2(ap):   # [P,1,R] -> [P,2,R]
        return ap.broadcast_to([P, 2, R])

    # multiplier chain (all tensor_tensor mult)
    tt = nc.vector.tensor_tensor
    # 1: t01 = gA * [x0,x1]            -> mm[0:2]
    tt(mv[:, 0:2, :], bc2(gv[:, 0:1, :]), xT[:, 0:2, :], op=mul)
    # 2: [mB,mC] = t01 * [g1,g2]       -> mm[0:2] (in place)
    tt(mv[:, 0:2, :], mv[:, 0:2, :], gv[:, 1:3, :], op=mul)
    # 3: t34 = mC * [x3,x4]            -> mm[2:4]
    tt(mv[:, 2:4, :], bc2(mv[:, 1:2, :]), xT[:, 3:5, :], op=mul)
    # 4: mF = w*g5 -> mm[4] ; mI = w*g8 -> mm[5]   (w = mm[3], read before op5)
    tt(mv[:, 4:5, :], mv[:, 3:4, :], gv[:, 5:6, :], op=mul)
    tt(mv[:, 5:6, :], mv[:, 3:4, :], gv[:, 8:9, :], op=mul)
    # 5: [mD,mE] = t34 * [g3,g4]       -> mm[2:4] (in place)
    tt(mv[:, 2:4, :], mv[:, 2:4, :], gv[:, 3:5, :], op=mul)
    # 6: t7 = mF * x7                  -> mm[6]
    tt(mv[:, 6:7, :], mv[:, 4:5, :], xT[:, 7:8, :], op=mul)
    # 7: [mG,mH] = t7 * [g6,g7]        -> mm[7:9]
    tt(mv[:, 7:9, :], bc2(mv[:, 6:7, :]), gv[:, 6:8, :], op=mul)
    # 8: t13 = gJ * x13                -> mm[9]
    tt(mv[:, 9:10, :], gv[:, 9:10, :], xT[:, 13:14, :], op=mul)
    # 9: mK = t13 * g10                -> mm[9] (in place)
    tt(mv[:, 9:10, :], mv[:, 9:10, :], gv[:, 10:11, :], op=mul)

    # seg index (step order) -> multiplier [P, R] slice
    m_of = {
        0: g[:, 0:R], 1: mm[:, 0:R], 2: mm[:, R:2 * R], 3: mm[:, 2 * R:3 * R],
        4: mm[:, 3 * R:4 * R], 5: mm[:, 4 * R:5 * R], 6: mm[:, 7 * R:8 * R],
        7: mm[:, 8 * R:9 * R], 8: mm[:, 5 * R:6 * R], 9: g[:, 9 * R:10 * R],
        10: mm[:, 9 * R:10 * R],
    }
    for k, (s, e) in enumerate(SEGS):
        w = e - s
        mb = m_of[k].unsqueeze(-1).broadcast_to([P, R, w])
        tt(x3[:, :, s:e], x3[:, :, s:e], mb, op=mul)

    dst = y[row0:row0 + P * R, :].rearrange("(p r) c -> p (r c)", p=P)
    nc.sync.dma_start(dst, xt[:, :])


def build_nc(rows_per_core):
    assert rows_per_core % P == 0
    nc = bacc.Bacc("TRN2", target_bir_lowering=False)
    x = nc.dram_tensor("x", [rows_per_core, NCOLS], F32, kind="ExternalInput")
    y = nc.dram_tensor("y", [rows_per_core, NCOLS], F32, kind="ExternalOutput")
    with TileContext(nc) as tc:
        with tc.tile_pool(name="xbuf", bufs=3) as xpool, \
             tc.tile_pool(name="stats", bufs=2) as spool, \
             tc.tile_pool(name="singles", bufs=1) as singles:
            bias_tiny = singles.tile([P, 1], F32)
            nc.vector.memset(bias_tiny[:, :], 1e-30)
            row0 = 0
            for R in _tile_splits(rows_per_core // P):
                _emit_tile(nc, tc, xpool, spool, x, y, row0, R, bias_tiny)
                row0 += P * R
    nc.finalize()
    return nc


_NC_CACHE = {}


def get_nc(rows_per_core):
    if rows_per_core not in _NC_CACHE:
        _NC_CACHE[rows_per_core] = build_nc(rows_per_core)
    return _NC_CACHE[rows_per_core]


def shard(x):
    """Pad rows to a multiple of 8*128 and split into 8 per-core shards."""
    n = x.shape[0]
    rpc = -(-n // (NCORES * P)) * P          # ceil to multiple of P
    total = rpc * NCORES
    if total > n:
        pad = np.ones((total - n, x.shape[1]), dtype=x.dtype)
        xp = np.concatenate([x, pad], axis=0)
    else:
        xp = x
    return [np.ascontiguousarray(xp[c * rpc:(c + 1) * rpc]) for c in range(NCORES)], rpc


def kernel(x):
    x = np.asarray(x, dtype=np.float32)
    n = x.shape[0]
    shards, rpc = shard(x)
    nc = get_nc(rpc)
    res = run_bass_kernel_spmd(nc, [{"x": s} for s in shards],
                               core_ids=list(range(NCORES)))
    out = np.concatenate([res.results[c]["y"] for c in range(NCORES)], axis=0)
    return out[:n]


def _make_jit(nc):
    """Build the same shard_map jit run_bass_via_pjrt builds, but reusable."""
    import jax
    from jax.sharding import Mesh, PartitionSpec, NamedSharding
    try:
        from jax.experimental.shard_map import shard_map
    except ImportError:  # newer jax
        from jax.shard_map import shard_map
    from concourse import bass2jax
    bass2jax.install_neuronx_cc_hook()

    partition_name = (nc.partition_id_tensor.name
                      if nc.partition_id_tensor else None)
    in_names, out_names, out_avals = [], [], []
    for alloc in nc.m.functions[0].allocations:
        if not isinstance(alloc, mybir.MemoryLocationSet):
            continue
        name = alloc.memorylocations[0].name
        if alloc.kind == "ExternalInput":
            if name != partition_name:
                in_names.append(name)
        elif alloc.kind == "ExternalOutput":
            out_names.append(name)
            out_avals.append(jax.core.ShapedArray(
                tuple(alloc.tensor_shape), mybir.dt.np(alloc.dtype)))
    n_params = len(in_names)
    all_names = in_names + out_names
    if partition_name is not None:
        all_names.append(partition_name)
    all_names = tuple(all_names)

    def _body(*args):
        operands = list(args)
        if partition_name is not None:
            operands.append(bass2jax.partition_id_tensor())
        outs = bass2jax._bass_exec_p.bind(
            *operands, out_avals=tuple(out_avals), in_names=all_names,
            out_names=tuple(out_names), lowering_input_output_aliases=(),
            sim_require_finite=True, sim_require_nnan=True, nc=nc)
        return tuple(outs)

    devices = jax.devices()[:NCORES]
    mesh = Mesh(np.asarray(devices), ("core",))
    nout = len(out_names)
    donate = tuple(range(n_params, n_params + nout))
    fn = jax.jit(
        shard_map(_body, mesh=mesh,
                  in_specs=(PartitionSpec("core"),) * (n_params + nout),
                  out_specs=(PartitionSpec("core"),) * nout,
                  check_rep=False),
        donate_argnums=donate, keep_unused=True)
    sharding = NamedSharding(mesh, PartitionSpec("core"))
    return fn, sharding, out_avals


def timed_exec_ns(x, iters=3):
    """Median wall time per on-device execution (ns), inputs device-resident."""
    import jax, time
    shards, rpc = shard(np.asarray(x, np.float32))
    nc = get_nc(rpc)
    fn, sharding, out_avals = _make_jit(nc)
    xg = jax.device_put(np.concatenate(shards, axis=0), sharding)
    zero_np = np.zeros((NCORES * out_avals[0].shape[0], *out_avals[0].shape[1:]),
                       out_avals[0].dtype)
    zsets = [jax.device_put(zero_np, sharding) for _ in range(iters + 1)]
    out = fn(xg, zsets[0])   # warmup / compile
    jax.block_until_ready(out)
    times = []
    for i in range(iters):
        t0 = time.perf_counter()
        out = fn(xg, zsets[i + 1])
        jax.block_until_ready(out)
        times.append(time.perf_counter() - t0)
    times.sort()
    return times[len(times) // 2] * 1e9
